# revision 1
# baseline (speedup 1.0000x reference)
"""AttentionFlow kernel for 8 TRN2 NeuronCores.

Sharding: data-parallel over batch B=8, one batch element per core, params
replicated. No collectives needed. Each core computes its full (C, 4D) output.

Per-core layout trick: everything downstream of the similarity matrix is
computed in TRANSPOSED ([feature, context]) layout so that mega's four
256-row feature blocks are directly usable as lhsT tiles of the final
(C,4D)@(4D,4D) matmul, and the C2Q attention probabilities come out of the
softmax already oriented as the lhsT of the u_tilde matmul.
"""

import numpy as np
import ml_dtypes

import concourse.bass as bass
import concourse.mybir as mybir
import concourse.tile as tile
from concourse import bacc
from concourse.bass_utils import run_bass_kernel_spmd
from concourse.masks import make_identity

B, C, Q, D = 8, 2048, 128, 256
F = 4 * D          # 1024
CT = C // 128      # 16 context tiles
FP32 = mybir.dt.float32
BF16 = mybir.dt.bfloat16
EXP = mybir.ActivationFunctionType.Exp

_cached = {}


def build_nc():
    nc = bacc.Bacc(None, target_bir_lowering=False, debug=False)

    q_ext = nc.declare_dram_parameter("q", [Q, D], BF16, isOutput=False)
    ctx_ext = nc.declare_dram_parameter("ctx", [C, D], BF16, isOutput=False)
    wsim_ext = nc.declare_dram_parameter("wsim", [128, 6], FP32, isOutput=False)
    w2t_ext = nc.declare_dram_parameter("w2t", [F, F], BF16, isOutput=False)
    b2_ext = nc.declare_dram_parameter("b2", [1, F], FP32, isOutput=False)
    out_ext = nc.declare_dram_parameter("out", [C, F], FP32, isOutput=True)

    with tile.TileContext(nc) as tc:
        with (
            tc.tile_pool(name="persist", bufs=1) as persist,
            tc.tile_pool(name="p1", bufs=3) as p1,
            tc.tile_pool(name="p3", bufs=3) as p3,
        ):
            # ---------------- persistent tiles ----------------
            q_bf = persist.tile([Q, D], BF16, name="q_bf", tag="q_bf")
            wsim = persist.tile([128, 6], FP32, name="wsim", tag="wsim")
            wsim_bf = persist.tile([128, 6], BF16, name="wsim_bf", tag="wsim_bf")
            w2t = persist.tile([128, 8, F], BF16, name="w2t", tag="w2t")
            b2_sb = persist.tile([128, F], FP32, name="b2_sb", tag="b2_sb")
            ctxT = [persist.tile([128, C], BF16, name=f"ctxT{h}", tag=f"ctxT{h}")
                    for h in range(2)]
            U_bf = [persist.tile([128, C], BF16, name=f"U{h}", tag=f"U{h}")
                    for h in range(2)]
            ctx_nat = [persist.tile([128, D], BF16, name=f"cn{i}", tag=f"cn{i}")
                       for i in range(CT)]
            qT_bf = persist.tile([128, D], BF16, name="qT_bf", tag="qT_bf")
            qmodT = persist.tile([128, D], BF16, name="qmodT", tag="qmodT")
            ident = persist.tile([128, 128], BF16, name="ident", tag="ident")
            sc_row = persist.tile([1, C], BF16, name="sc_row", tag="sc_row")
            sq_row = persist.tile([1, 128], BF16, name="sq_row", tag="sq_row")
            ones_col = persist.tile([128, 1], BF16, name="ones_c", tag="ones_c")
            ones_row = persist.tile([1, 128], BF16, name="ones_r", tag="ones_r")
            nm_coll = persist.tile([128, CT], FP32, name="nm_coll", tag="nm_coll")
            e_coll = persist.tile([128, CT], BF16, name="e_coll", tag="e_coll")
            h_col = persist.tile([128, 2], FP32, name="h_col", tag="h_col")
            invZb = persist.tile([128, 1], FP32, name="invZb", tag="invZb")

            # ---------------- prologue DMAs ----------------
            nc.sync.dma_start(q_bf[:], q_ext[:, :])
            nc.sync.dma_start(wsim[:], wsim_ext[:, :])
            nc.sync.dma_start(b2_sb[:], b2_ext[0:1, :].to_broadcast((128, F)))
            for t in range(8):
                nc.sync.dma_start(w2t[:, t], w2t_ext[t * 128:(t + 1) * 128, :])
            for h in range(2):
                nc.sync.dma_start_transpose(
                    ctxT[h][:], ctx_ext[:, h * 128:(h + 1) * 128]
                )
            for i in range(CT):
                nc.sync.dma_start(
                    ctx_nat[i][:], ctx_ext[i * 128:(i + 1) * 128, :]
                )

            make_identity(nc, ident[:])
            nc.gpsimd.memset(ones_col[:], 1.0)
            nc.gpsimd.memset(ones_row[:], 1.0)
            nc.vector.tensor_copy(wsim_bf[:], wsim[:])

            with tc.tile_pool(name="p1ps", bufs=1, space="PSUM") as p1ps:
                # q^T and qmod^T  (2 PE transposes of q halves)
                for h in range(2):
                    hs = slice(h * 128, (h + 1) * 128)
                    tp = p1ps.tile([128, 128], BF16, name=f"tp{h}",
                                   tag="misc", bufs=2)
                    nc.tensor.transpose(tp[:], q_bf[:, hs], ident[:])
                    nc.scalar.copy(qT_bf[:, hs], tp[:])
                    nc.vector.tensor_scalar_mul(
                        qmodT[:, hs], qT_bf[:, hs], wsim[:, 4 + h:5 + h]
                    )

                # sq row
                sq_ps = p1ps.tile([1, 128], FP32, name="sq_ps", tag="misc", bufs=2)
                for h in range(2):
                    nc.tensor.matmul(
                        sq_ps[:], wsim_bf[:, 2 + h:3 + h],
                        qT_bf[:, h * 128:(h + 1) * 128],
                        start=(h == 0), stop=(h == 1),
                    )
                nc.scalar.copy(sq_row[:], sq_ps[:])

                # sc row:  sc[1, c] = sum_d wc[d] * ctxT[d, c]
                for j in range(4):
                    sc_ps = p1ps.tile([1, 512], FP32, name=f"sc{j}",
                                      tag="misc", bufs=2)
                    for h in range(2):
                        nc.tensor.matmul(
                            sc_ps[:], wsim_bf[:, h:h + 1],
                            ctxT[h][:, j * 512:(j + 1) * 512],
                            start=(h == 0), stop=(h == 1),
                        )
                    nc.scalar.copy(sc_row[0:1, j * 512:(j + 1) * 512], sc_ps[:])

                # ---------------- phase 1: per c-tile C2Q attention --------
                for i in range(CT):
                    cs = slice(i * 128, (i + 1) * 128)

                    # sim[c, q] = ctx . (q*wcq) + sc[c] + sq[q]
                    sim_ps = p1ps.tile([128, 128], FP32, name=f"sim{i}",
                                       tag="sim", bufs=2)
                    for h in range(2):
                        nc.tensor.matmul(
                            sim_ps[:], ctxT[h][:, cs],
                            qmodT[:, h * 128:(h + 1) * 128],
                            start=(h == 0), stop=False,
                        )
                    nc.tensor.matmul(
                        sim_ps[:], sc_row[:, cs], ones_row[:],
                        start=False, stop=False,
                    )
                    nc.tensor.matmul(
                        sim_ps[:], ones_row[:], sq_row[:],
                        start=False, stop=True,
                    )

                    # softmax over q (free dim); nm = -max
                    nc.vector.reduce_max(
                        nm_coll[:, i:i + 1], sim_ps[:],
                        axis=mybir.AxisListType.X, negate=True,
                    )
                    p_bf = p1.tile([128, 128], BF16, name=f"p{i}", tag="p")
                    nc.scalar.activation(
                        p_bf[:], sim_ps[:], EXP,
                        bias=nm_coll[:, i:i + 1], scale=1.0,
                    )
                    se = p1.tile([128, 1], FP32, name=f"se{i}", tag="se")
                    nc.vector.reduce_sum(se[:], p_bf[:], axis=mybir.AxisListType.X)
                    inv_se = p1.tile([128, 1], FP32, name=f"ise{i}", tag="ise")
                    nc.vector.reciprocal(inv_se[:], se[:])
                    a_bf = p1.tile([128, 128], BF16, name=f"a{i}", tag="a")
                    nc.vector.tensor_scalar_mul(a_bf[:], p_bf[:], inv_se[:])

                    aT_ps = p1ps.tile([128, 128], BF16, name=f"aTp{i}",
                                      tag="aT", bufs=2)
                    nc.tensor.transpose(aT_ps[:], a_bf[:], ident[:])
                    aT_bf = p1.tile([128, 128], BF16, name=f"aTs{i}", tag="aTs")
                    nc.scalar.copy(aT_bf[:], aT_ps[:])

                    u_ps = p1ps.tile([128, 256], FP32, name=f"u{i}",
                                     tag="u", bufs=2)
                    for h in range(2):
                        nc.tensor.matmul(
                            u_ps[:, h * 128:(h + 1) * 128],
                            q_bf[:, h * 128:(h + 1) * 128], aT_bf[:],
                            start=True, stop=True,
                        )
                    for h in range(2):
                        nc.vector.tensor_copy(
                            U_bf[h][:, cs], u_ps[:, h * 128:(h + 1) * 128]
                        )

                # ------------- phase 2: Q2C softmax over all c -------------
                nc.scalar.activation(e_coll[:], nm_coll[:], EXP,
                                     bias=0.0, scale=-1.0)
                zp = p1ps.tile([1, CT], FP32, name="zp", tag="misc", bufs=2)
                nc.tensor.matmul(zp[:], ones_col[:], e_coll[:],
                                 start=True, stop=True)
                zs = p1.tile([1, 1], FP32, name="zs", tag="zs")
                nc.vector.reduce_sum(zs[:], zp[:], axis=mybir.AxisListType.X)
                invz = p1.tile([1, 1], FP32, name="invz", tag="iz")
                nc.vector.reciprocal(invz[:], zs[:])
                invz_bf = p1.tile([1, 1], BF16, name="invz_bf", tag="izb")
                nc.vector.tensor_copy(invz_bf[:], invz[:])
                izb_ps = p1ps.tile([128, 1], FP32, name="izb_ps",
                                   tag="misc", bufs=2)
                nc.tensor.matmul(izb_ps[:], ones_row[:], invz_bf[:],
                                 start=True, stop=True)
                nc.scalar.copy(invZb[:], izb_ps[:])

                # H[d, 2cols] = sum_c ctx[c, d] e[c]
                h_ps = p1ps.tile([128, 2], FP32, name="h_ps", tag="misc", bufs=2)
                for h in range(2):
                    for i in range(CT):
                        nc.tensor.matmul(
                            h_ps[:, h:h + 1],
                            ctx_nat[i][:, h * 128:(h + 1) * 128],
                            e_coll[:, i:i + 1],
                            start=(i == 0), stop=(i == CT - 1),
                        )
                nc.vector.tensor_scalar_mul(h_col[:], h_ps[:], invZb[:])

            # ---------------- phase 3: g = mega @ W2T + b2 ----------------
            with tc.tile_pool(name="p3ps", bufs=3, space="PSUM") as p3ps:
                for i in range(CT):
                    cs = slice(i * 128, (i + 1) * 128)
                    m2 = [p3.tile([128, 128], BF16, name=f"m2_{h}_{i}",
                                  tag=f"m2_{h}") for h in range(2)]
                    m3 = [p3.tile([128, 128], BF16, name=f"m3_{h}_{i}",
                                  tag=f"m3_{h}") for h in range(2)]
                    for h in range(2):
                        nc.vector.tensor_mul(m2[h][:], U_bf[h][:, cs],
                                             ctxT[h][:, cs])
                        nc.gpsimd.tensor_scalar_mul(
                            m3[h][:], ctxT[h][:, cs], h_col[:, h:h + 1]
                        )
                    lhs_tiles = [
                        ctxT[0][:, cs], ctxT[1][:, cs],
                        U_bf[0][:, cs], U_bf[1][:, cs],
                        m2[0][:], m2[1][:],
                        m3[0][:], m3[1][:],
                    ]
                    g_ps = [p3ps.tile([128, 512], FP32, name=f"g{j}_{i}",
                                      tag=f"g{j}", bufs=3) for j in range(2)]
                    for f in range(8):
                        for j in range(2):
                            nc.tensor.matmul(
                                g_ps[j][:],
                                lhs_tiles[f],
                                w2t[:, f, j * 512:(j + 1) * 512],
                                start=(f == 0), stop=(f == 7),
                            )
                    g_sb = p3.tile([128, F], FP32, name=f"g_sb{i}", tag="g_sb")
                    for j in range(2):
                        nc.vector.tensor_tensor(
                            g_sb[:, j * 512:(j + 1) * 512], g_ps[j][:],
                            b2_sb[:, j * 512:(j + 1) * 512],
                            mybir.AluOpType.add,
                        )
                    nc.sync.dma_start(out_ext[cs, :], g_sb[:])

    nc.finalize()
    return nc


def kernel(questions, contexts, questions_mask, contexts_mask, w_sim, W2, b2):
    if "nc" not in _cached:
        _cached["nc"] = build_nc()
    nc = _cached["nc"]

    bf16 = ml_dtypes.bfloat16
    questions = np.asarray(questions, dtype=np.float32)
    contexts = np.asarray(contexts, dtype=np.float32)
    W2 = np.asarray(W2, dtype=np.float32)
    w2t = np.ascontiguousarray(W2.T).astype(bf16)
    wsim_cols = np.ascontiguousarray(
        np.asarray(w_sim, dtype=np.float32).reshape(6, 128).T
    )

    b2f = np.asarray(b2, dtype=np.float32).reshape(1, F)
    in_maps = []
    for i in range(B):
        in_maps.append({
            "q": np.asarray(questions[i]).astype(bf16),
            "ctx": np.asarray(contexts[i]).astype(bf16),
            "wsim": wsim_cols,
            "w2t": w2t,
            "b2": b2f,
        })
    res = run_bass_kernel_spmd(nc, in_maps, core_ids=list(range(B)))
    out = np.stack([res.results[i]["out"] for i in range(B)], axis=0)
    return out.astype(np.float32)



# revision 4
# speedup vs baseline: 1.5582x; 1.5582x over previous
"""AttentionFlow kernel for 8 TRN2 NeuronCores.

Sharding: data-parallel over batch B=8, one batch element per core, params
replicated. No collectives.

Per-core algorithm (C=2048 contexts, Q=128 queries, D=256, F=4D=1024):

  sim[c,q] = ctx.(wcq*q) + sc[c] + sq[q]
  a = softmax_q(sim); u = a@q; bw = softmax_c(max_q sim); h = bw@ctx
  g = [ctx, u, u*ctx, h*ctx] @ W2^T + b2

Key restructurings vs the naive flow:
  * sc folded into the sim matmul moving operand: qmodc = wcq*q^T + wc
    (adding wc[d] to every q-column adds sc[c] to every sim row).
  * h is constant over c, so the h*ctx block folds into the weights:
    afold = W2^T[0:256] + h * W2^T[768:1024]; the ctx@afold matmul
    covers both the ctx and h*ctx mega blocks.
  * u has rank <= Q, so u @ W2^T[256:512] = a @ (q @ W2^T[256:512]);
    qB = q @ B is precomputed once (128x1024), replacing a 2048x256
    contraction with a 2048x128 one.
  * Phase 3 computes g^T[o,c] with the weight blocks stationary in the
    PE array and the [feature, c]-layout data streaming, accumulating
    5 K-blocks per (o,c) tile. Output is written transposed (bf16) and
    fixed up on host (transpose + b2 add), halving store traffic.
  * ctx is DMA'd once in natural layout; ctxT is built with PE
    transposes on the fly (the DMA-transpose descriptor storm cost the
    old kernel ~20us of prologue).
  * exp() uses activation accum_out to produce softmax row sums free.
"""

import numpy as np
import ml_dtypes

import concourse.bass as bass
import concourse.mybir as mybir
import concourse.tile as tile
from concourse import bacc
from concourse.bass_utils import run_bass_kernel_spmd
from concourse.masks import make_identity

B, C, Q, D = 8, 2048, 128, 256
F = 4 * D          # 1024
CT = C // 128      # 16 context tiles
NCH = CT // 4      # 4 chunks of 4 tiles
FP32 = mybir.dt.float32
BF16 = mybir.dt.bfloat16
EXP = mybir.ActivationFunctionType.Exp
ADD = mybir.AluOpType.add
MULT = mybir.AluOpType.mult
AXX = mybir.AxisListType.X

_cached = {}


def build_nc():
    nc = bacc.Bacc(None, target_bir_lowering=False, debug=False)

    q_ext = nc.declare_dram_parameter("q", [Q, D], BF16, isOutput=False)
    ctx_ext = nc.declare_dram_parameter("ctx", [C, D], BF16, isOutput=False)
    wsim_ext = nc.declare_dram_parameter("wsim", [128, 6], FP32, isOutput=False)
    w2t_ext = nc.declare_dram_parameter("w2t", [F, F], BF16, isOutput=False)
    out_ext = nc.declare_dram_parameter("out", [F, C], BF16, isOutput=True)

    with tile.TileContext(nc) as tc:
        with (
            tc.tile_pool(name="persist", bufs=1) as persist,
            tc.tile_pool(name="work", bufs=2) as work,
        ):
            # ---------------- persistent tiles ----------------
            q_bf = persist.tile([Q, D], BF16, name="q_bf", tag="q_bf")
            wsim = persist.tile([128, 6], FP32, name="wsim", tag="wsim")
            w2t = persist.tile([128, 8, F], BF16, name="w2t", tag="w2t")
            ident = persist.tile([128, 128], BF16, name="ident", tag="ident")
            qT = persist.tile([128, D], BF16, name="qT", tag="qT")
            qmodc = persist.tile([128, D], BF16, name="qmodc", tag="qmodc")
            sq_row = persist.tile([1, 128], BF16, name="sq_row", tag="sq_row")
            ones_row = persist.tile([1, 128], BF16, name="ones_r", tag="ones_r")
            ones_col = persist.tile([128, 1], BF16, name="ones_c", tag="ones_c")
            one_bf = persist.tile([1, 1], BF16, name="one_bf", tag="one_bf")
            ctxT = [persist.tile([128, C], BF16, name=f"ctxT{h}", tag=f"ctxT{h}")
                    for h in range(2)]
            ctx_nat = [persist.tile([128, D], BF16, name=f"cn{i}", tag=f"cn{i}")
                       for i in range(CT)]
            AT = persist.tile([128, C], BF16, name="AT", tag="AT")
            M2 = [persist.tile([128, C], BF16, name=f"M2{h}", tag=f"M2{h}")
                  for h in range(2)]
            qB = persist.tile([128, F], BF16, name="qB", tag="qB")
            afold = persist.tile([128, 2, F], BF16, name="afold", tag="afold")
            nm_coll = persist.tile([128, CT], FP32, name="nm_coll", tag="nm_coll")
            e_coll = persist.tile([128, CT], BF16, name="e_coll", tag="e_coll")
            h_col = persist.tile([128, 2], FP32, name="h_col", tag="h_col")

            # ---------------- DMAs (ordered for early availability) -----
            nc.sync.dma_start(q_bf[:], q_ext[:, :])
            nc.sync.dma_start(wsim[:], wsim_ext[:, :])
            for i in range(2):
                nc.sync.dma_start(ctx_nat[i][:], ctx_ext[i * 128:(i + 1) * 128, :])
            for t in (2, 3):   # qB needs these first
                nc.sync.dma_start(w2t[:, t], w2t_ext[t * 128:(t + 1) * 128, :])
            for i in range(2, CT):
                nc.sync.dma_start(ctx_nat[i][:], ctx_ext[i * 128:(i + 1) * 128, :])
            for t in (4, 5, 0, 1, 6, 7):
                nc.sync.dma_start(w2t[:, t], w2t_ext[t * 128:(t + 1) * 128, :])

            make_identity(nc, ident[:])
            nc.vector.memset(ones_row[:], 1.0)
            nc.vector.memset(ones_col[:], 1.0)
            nc.vector.memset(one_bf[:], 1.0)

            # ---------------- prologue: q^T, qmodc, sq ----------------
            with tc.tile_pool(name="pre_ps", bufs=1, space="PSUM") as pps:
                wsim_bf = work.tile([128, 6], BF16, name="wsim_bf", tag="wsb")
                nc.vector.tensor_copy(wsim_bf[:], wsim[:])
                for h in range(2):
                    hs = slice(h * 128, (h + 1) * 128)
                    tp = pps.tile([128, 128], BF16, name=f"qtp{h}", tag="qtp",
                                  bufs=2)
                    nc.tensor.transpose(tp[:], q_bf[:, hs], ident[:])
                    nc.scalar.copy(qT[:, hs], tp[:])
                    # qmodc = wcq*qT + wc  (folds the sc rank-1 into sim)
                    nc.vector.tensor_scalar(
                        qmodc[:, hs], qT[:, hs],
                        wsim[:, 4 + h:5 + h], wsim[:, h:h + 1],
                        MULT, ADD,
                    )
                sq_ps = pps.tile([1, 128], FP32, name="sq_ps", tag="sqp", bufs=1)
                for h in range(2):
                    nc.tensor.matmul(
                        sq_ps[:], wsim_bf[:, 2 + h:3 + h],
                        qT[:, h * 128:(h + 1) * 128],
                        start=(h == 0), stop=(h == 1),
                    )
                nc.scalar.copy(sq_row[:], sq_ps[:])

            # ---------------- phase 1 (lag-2 software pipeline) --------
            with tc.tile_pool(name="p1ps", bufs=1, space="PSUM") as p1ps:
                h_ps = p1ps.tile([1, D], FP32, name="h_ps", tag="hps", bufs=1)
                sim_ps = {}
                a_bf = {}

                def produce(i):
                    cs = slice(i * 128, (i + 1) * 128)
                    # ctxT tiles via PE transpose of the natural-layout DMA
                    for h in range(2):
                        hs = slice(h * 128, (h + 1) * 128)
                        ctp = p1ps.tile([128, 128], BF16, name=f"ctp{i}_{h}",
                                        tag="ctp", bufs=2)
                        nc.tensor.transpose(ctp[:], ctx_nat[i][:, hs], ident[:])
                        if h == 0:
                            nc.vector.tensor_copy(ctxT[h][:, cs], ctp[:])
                        else:
                            nc.scalar.copy(ctxT[h][:, cs], ctp[:])
                    # sim[c, q] (+sc via qmodc), then +sq rank-1
                    sp = p1ps.tile([128, 128], FP32, name=f"sim{i}",
                                   tag="sim", bufs=2)
                    sim_ps[i] = sp
                    for h in range(2):
                        nc.tensor.matmul(
                            sp[:], ctxT[h][:, cs],
                            qmodc[:, h * 128:(h + 1) * 128],
                            start=(h == 0), stop=False,
                        )
                    nc.tensor.matmul(sp[:], ones_row[:], sq_row[:],
                                     start=False, stop=True)
                    # softmax stats
                    nc.vector.reduce_max(nm_coll[:, i:i + 1], sp[:],
                                         axis=AXX, negate=True)
                    nc.scalar.activation(e_coll[:, i:i + 1],
                                         nm_coll[:, i:i + 1], EXP,
                                         bias=0.0, scale=-1.0)
                    p_bf = work.tile([128, 128], BF16, name=f"p{i}", tag="p",
                                     bufs=3)
                    se = work.tile([128, 1], FP32, name=f"se{i}", tag="se",
                                   bufs=3)
                    nc.scalar.activation(p_bf[:], sp[:], EXP,
                                         bias=nm_coll[:, i:i + 1], scale=1.0,
                                         accum_out=se[:])
                    ise = work.tile([128, 1], FP32, name=f"ise{i}", tag="ise",
                                    bufs=3)
                    nc.vector.reciprocal(ise[:], se[:])
                    ab = work.tile([128, 128], BF16, name=f"a{i}", tag="a",
                                   bufs=3)
                    a_bf[i] = ab
                    nc.vector.tensor_scalar_mul(ab[:], p_bf[:], ise[:])

                def consume(j):
                    cs = slice(j * 128, (j + 1) * 128)
                    aT = p1ps.tile([128, 128], BF16, name=f"aT{j}",
                                   tag="aT", bufs=1)
                    nc.tensor.transpose(aT[:], a_bf[j][:], ident[:])
                    nc.scalar.copy(AT[:, cs], aT[:])
                    # Q2C numerator: h_ps[1, d] += e[c] * ctx[c, d]
                    nc.tensor.matmul(h_ps[:], e_coll[:, j:j + 1],
                                     ctx_nat[j][:],
                                     start=(j == 0), stop=(j == CT - 1))
                    del a_bf[j], sim_ps[j]

                def u_chunk(ch):
                    c4 = slice(ch * 512, (ch + 1) * 512)
                    for h in range(2):
                        up = p1ps.tile([128, 512], FP32, name=f"u{ch}_{h}",
                                       tag="u", bufs=1)
                        nc.tensor.matmul(
                            up[:], q_bf[:, h * 128:(h + 1) * 128], AT[:, c4],
                            start=True, stop=True,
                        )
                        uc = work.tile([128, 512], BF16, name=f"uc{ch}_{h}",
                                       tag="uc", bufs=2)
                        nc.vector.tensor_copy(uc[:], up[:])
                        nc.vector.tensor_mul(M2[h][:, c4], uc[:],
                                             ctxT[h][:, c4])

                def emit_qB():
                    for j in range(2):
                        js = slice(j * 512, (j + 1) * 512)
                        qp = p1ps.tile([128, 512], FP32, name=f"qb{j}",
                                       tag="qb", bufs=1)
                        for h in range(2):
                            nc.tensor.matmul(
                                qp[:], qT[:, h * 128:(h + 1) * 128],
                                w2t[:, 2 + h, js],
                                start=(h == 0), stop=(h == 1),
                            )
                        nc.vector.tensor_copy(qB[:, js], qp[:])

                for ii in range(CT + 2):
                    if ii < CT:
                        produce(ii)
                    if ii == 4:
                        emit_qB()
                    jj = ii - 2
                    if jj >= 0:
                        consume(jj)
                        if jj % 4 == 3:
                            u_chunk(jj // 4)

            # ---------------- phase 2: Q2C normalization + weight fold --
            with tc.tile_pool(name="p2ps", bufs=1, space="PSUM") as p2ps:
                zp = p2ps.tile([1, CT], FP32, name="zp", tag="zp", bufs=1)
                nc.tensor.matmul(zp[:], ones_col[:], e_coll[:],
                                 start=True, stop=True)
                zs = work.tile([1, 1], FP32, name="zs", tag="zs")
                nc.vector.reduce_sum(zs[:], zp[:], axis=AXX)
                invz = work.tile([1, 1], FP32, name="invz", tag="iz")
                nc.vector.reciprocal(invz[:], zs[:])
                h_sb = work.tile([1, D], FP32, name="h_sb", tag="hsb")
                nc.scalar.copy(h_sb[:], h_ps[:])
                h_bf = work.tile([1, D], BF16, name="h_bf", tag="hbf")
                nc.vector.tensor_scalar_mul(h_bf[:], h_sb[:], invz[:])
                hc = p2ps.tile([128, 2], FP32, name="hc", tag="hc", bufs=1)
                for h in range(2):
                    nc.tensor.matmul(hc[:, h:h + 1],
                                     h_bf[:, h * 128:(h + 1) * 128],
                                     one_bf[:], start=True, stop=True)
                nc.scalar.copy(h_col[:], hc[:])
                for h in range(2):
                    hD = work.tile([128, F], BF16, name=f"hD{h}", tag="hD")
                    nc.vector.tensor_scalar_mul(hD[:], w2t[:, 6 + h, :],
                                                h_col[:, h:h + 1])
                    nc.vector.tensor_tensor(afold[:, h, :], w2t[:, h, :],
                                            hD[:], ADD)

            # ---------------- phase 3: g^T = sum_k Wk^T @ megaT ----------
            with tc.tile_pool(name="p3ps", bufs=1, space="PSUM") as p3ps:
                for ob in range(8):
                    obs = slice(ob * 128, (ob + 1) * 128)
                    g_ps = [p3ps.tile([128, 512], FP32, name=f"g{ob}_{cj}",
                                      tag="g", bufs=8) for cj in range(4)]
                    ks = [
                        (qB[:, obs], AT),
                        (w2t[:, 4, obs], M2[0]),
                        (w2t[:, 5, obs], M2[1]),
                        (afold[:, 0, obs], ctxT[0]),
                        (afold[:, 1, obs], ctxT[1]),
                    ]
                    for k, (lhs, rhs) in enumerate(ks):
                        for cj in range(4):
                            nc.tensor.matmul(
                                g_ps[cj][:], lhs,
                                rhs[:, cj * 512:(cj + 1) * 512],
                                start=(k == 0), stop=(k == len(ks) - 1),
                            )
                    gt = work.tile([128, C], BF16, name=f"gt{ob}", tag="gt",
                                   bufs=2)
                    for cj in range(4):
                        nc.vector.tensor_copy(gt[:, cj * 512:(cj + 1) * 512],
                                              g_ps[cj][:])
                    nc.sync.dma_start(out_ext[obs, :], gt[:])

    nc.finalize()
    return nc


def kernel(questions, contexts, questions_mask, contexts_mask, w_sim, W2, b2):
    if "nc" not in _cached:
        _cached["nc"] = build_nc()
    nc = _cached["nc"]

    bf16 = ml_dtypes.bfloat16
    questions = np.asarray(questions, dtype=np.float32)
    contexts = np.asarray(contexts, dtype=np.float32)
    W2 = np.asarray(W2, dtype=np.float32)
    w2t = np.ascontiguousarray(W2.T).astype(bf16)
    wsim_cols = np.ascontiguousarray(
        np.asarray(w_sim, dtype=np.float32).reshape(6, 128).T
    )

    in_maps = []
    for i in range(B):
        in_maps.append({
            "q": np.asarray(questions[i]).astype(bf16),
            "ctx": np.asarray(contexts[i]).astype(bf16),
            "wsim": wsim_cols,
            "w2t": w2t,
        })
    res = run_bass_kernel_spmd(nc, in_maps, core_ids=list(range(B)))
    _cached["last_res"] = res
    b2f = np.asarray(b2, dtype=np.float32)
    out = np.stack(
        [res.results[i]["out"].astype(np.float32).T + b2f[None, :]
         for i in range(B)], axis=0)
    return out
